# revision 14
# baseline (speedup 1.0000x reference)
"""NeuralMMU Trainium2 kernel, v2 — transposed second layer.

Per core (131072 addrs), 87 iterations sized [512, 1024, 84x1536, 512]
(small first iterations so the first Gelu starts early; small final
iteration so the pipeline drain is short).

  1. Host sends bit planes as bf16 [96, 131072] (bit k of addr a at
     partition k, replicated 3x for the 3-way bf16 split of W1). A
     single `boot` DMA carries W1+b1 plus iteration-0's bits so one
     DMA chain gates the first L1/Gelu; input groups are single
     iterations during ramp-up, pairs afterwards, prefetched ~6
     iterations ahead through 4 rotating buffers.
  2. L1: bf16 matmuls k=96 (512-addr blocks): bits @ (W1hi;W1mid;W1lo)
     -> PSUM hpre [128, <=1536] (exact: bits are 0/1, f32 accumulate).
  3. ACT Gelu(+b1): PSUM -> SBUF h f32, one instr/iter. This is the
     modeled bottleneck (~0.83 ns/elem + ~185 ns init per instr,
     ~126.6 us busy); everything else hides behind it.
  4. L2 TRANSPOSED: per 128-addr chunk, matmul with the h chunk
     [128 hid, 128 addr] as the *stationary* operand and W2[:, :26] f32
     as the *moving* operand -> PSUM pk [128 addr, nch*26] f32, exact,
     all chunks in one PSUM bank (start on first chunk, stop on last).
     Model cost 26*4 cyc/chunk vs 512*4 cyc per 512 addrs when h is the
     moving side: ~4.7x less PE time for the heavy layer. (LDWEIGHTS
     per chunk is free in the cost model and pipelined on silicon.)
  5. DVE: is_gt vs replicated thresholds (0.5 - b2[k]) -> bf16 bits,
     multiply by replicated 2^i weights (i = bit index within the lo/hi
     13-bit half), 4D tensor_reduce -> [128, 2*nch] f32 (lo, hi).
  6. Output batched 16 iters per DMA + a singleton final batch; host
     combines lo + 8192*hi.

PSUM: hpre 2 bufs x 3 banks + pk 2 bufs x 1 bank = 8 banks exactly.
(CHUNK=1664 by sharing pk into hpre's 4th bank was tried and is ~44 us
SLOWER: the bank's zero-region start=True forces a Tile dependency of
the next L1 on pk's DVE reader, serializing ACT behind DVE.)
Cost model (graded metric) 135.5 us vs 299.4 us baseline; correctness
runs on real silicon via PJRT (1/1048576 borderline mismatch, same as
the baseline, rel err 0.0065).
"""

import numpy as np
from contextlib import ExitStack

import concourse.bass as bass
import concourse.mybir as mybir
import concourse.tile as tile
from concourse import bacc, bass_utils

B = 1_048_576
NCORES = 8
PER = B // NCORES            # 131072 addrs per core
BLK = 512                    # addrs per L1 matmul block
CHUNK = 1536                 # max addrs per iteration (3 PSUM banks)
MMCH = 128                   # addrs per transposed L2 matmul
NBITS = 26

SIZES = [512, 1024] + [1536] * 84 + [512]
assert sum(SIZES) == PER
N_ITERS = len(SIZES)         # 87
CSTART = [0]
for _s in SIZES:
    CSTART.append(CSTART[-1] + _s)

# Input DMA groups (lists of iterations): singles during ramp-up, pairs
# after. Iteration 0's bits ride in the boot tensor, not in a group.
GROUPS = [[1], [2], [3]] + [[i, i + 1] for i in range(4, 86, 2)] + [[86]]
assert [t for g in GROUPS for t in g] == list(range(1, N_ITERS))
GRP_OF = {}
for _gi, _g in enumerate(GROUPS):
    for _t in _g:
        GRP_OF[_t] = _gi

# Output DMA batches: sixteen iterations each, then the tail alone so
# the final DMA after the last compute is tiny.
OBATCH = [list(range(r, min(r + 16, 86))) for r in range(0, 86, 16)] + [[86]]
OB_OF = {}
OB_OFF = {}
OB_USED = []
for _bi, _b in enumerate(OBATCH):
    used = 0
    for _t in _b:
        OB_OF[_t] = _bi
        OB_OFF[_t] = used
        used += 2 * (SIZES[_t] // MMCH)
    OB_USED.append(used)
NOUT = len(OBATCH)
OUTW = 24 * 16

F32 = mybir.dt.float32
BF16 = mybir.dt.bfloat16
AF = mybir.ActivationFunctionType
ALU = mybir.AluOpType
AX = mybir.AxisListType

# cst column layout (f32 columns); part A (w1b + b1) is DMA'd first so
# L1/Gelu can start before the larger part B arrives.
# boot tensor: w1b + b1 + iteration-0 bit planes, fetched in ONE DMA so a
# single 5-stage DMA chain gates the first L1/Gelu.
BT_W1 = 0         # [128, 64] f32 = [128, 128] bf16 3-way W1 split
BT_B1 = 64        # [128, 1] f32
BT_BITS = 65      # [96, 256] f32 = [96, 512] bf16 iteration-0 bits
BT_TOT = 65 + 256
# cst tensor (rest of the constants)
C_W2 = 0          # [128, 26] f32
C_TR = 26         # [128, 312] f32 thresholds (0.5 - b2[k]) replicated x12
C_WR = 338        # [128, 156] f32 = [128, 312] bf16 pack weights 2^i
C_TOT = 494


def build_nc() -> bass.Bass:
    nc = bacc.Bacc("TRN2")

    bp = nc.dram_tensor("bp", [96, PER], BF16, kind="ExternalInput")
    boot_d = nc.dram_tensor("boot", [128, BT_TOT], F32, kind="ExternalInput")
    cst_d = nc.dram_tensor("cst", [128, C_TOT], F32, kind="ExternalInput")
    outp = nc.dram_tensor("outp", [NOUT, 128, OUTW], F32, kind="ExternalOutput")

    with ExitStack() as ctx:
        tc = ctx.enter_context(tile.TileContext(nc))
        const = ctx.enter_context(tc.tile_pool(name="const", bufs=1))
        rpool = ctx.enter_context(tc.tile_pool(name="rp", bufs=4))
        hpre_p = ctx.enter_context(tc.tile_pool(name="hpre", bufs=2, space="PSUM"))
        hp = ctx.enter_context(tc.tile_pool(name="hp", bufs=3))
        pkp = ctx.enter_context(tc.tile_pool(name="pkp", bufs=2, space="PSUM"))
        yp = ctx.enter_context(tc.tile_pool(name="yp", bufs=2))
        zp = ctx.enter_context(tc.tile_pool(name="zp", bufs=2))
        so_p = ctx.enter_context(tc.tile_pool(name="so", bufs=2))

        boot = const.tile([128, BT_TOT], F32)
        nc.sync.dma_start(boot[:], boot_d[:])
        cst = const.tile([128, C_TOT], F32)

        w1b = boot[:, BT_W1:BT_W1 + 64].bitcast(BF16)    # [128, 128] bf16
        b1c = boot[:, BT_B1:BT_B1 + 1]
        bits0 = boot[0:96, BT_BITS:BT_BITS + 256].bitcast(BF16)  # [96, 512]
        w2s = cst[:, C_W2:C_W2 + NBITS]                  # [128, 26] f32
        trep = cst[:, C_TR:C_TR + 312]                   # [128, 312] f32
        wrep = cst[:, C_WR:C_WR + 156].bitcast(BF16)     # [128, 312] bf16

        R = [None, None, None, None]
        next_group = 0

        def load_group(gi):
            g = GROUPS[gi]
            lo, hi = CSTART[g[0]], CSTART[g[-1] + 1]
            Rg = rpool.tile([96, 2 * CHUNK], BF16, name="Rg", tag="R")
            nc.sync.dma_start(Rg[:, : hi - lo], bp[:, lo:hi])
            R[gi % 4] = Rg

        def prefetch(upto_iter):
            nonlocal next_group
            while (next_group < len(GROUPS)
                   and GROUPS[next_group][0] <= upto_iter):
                load_group(next_group)
                next_group += 1

        def bits_of(t):
            gi = GRP_OF[t]
            off = CSTART[t] - CSTART[GROUPS[gi][0]]
            return R[gi % 4][0:96, off:off + SIZES[t]]

        def l1mm(t, bits=None):
            na = SIZES[t]
            hpre = hpre_p.tile([128, CHUNK], F32, name="hpre", tag="hpre")
            if bits is None:
                bits = bits_of(t)
            for b in range(0, na, BLK):
                w = min(BLK, na - b)
                nc.tensor.matmul(
                    hpre[:, b:b + w],
                    w1b[0:96, :],
                    bits[:, b:b + w],
                    start=True, stop=True, tile_position=(0, 0),
                )
            return hpre

        # Startup: boot already issued; then bits for iterations 1-2, the
        # remaining constants, and iteration 3; in-loop prefetch takes over.
        prefetch(2)
        nc.sync.dma_start(cst[:], cst_d[:])
        prefetch(3)
        # Defer each iteration's entire transposed-L2 to the next
        # iteration's PE slot, and put L1(t+1) FIRST there: the in-order PE
        # queue then completes L1(t+1) ~880ns into Gelu(t)'s 1465ns window
        # instead of ~40ns after it (which cost a sem+decode stall on every
        # Gelu). h is triple-buffered so the deferred L2'(t-1) read never
        # collides with Gelu(t+1)'s output slot.
        def emit_l2(st, c0, c1):
            pk, h, nch = st["pk"], st["h"], st["nch"]
            for c in range(c0, min(c1, nch)):
                nc.tensor.matmul(
                    pk[:, NBITS * c:NBITS * (c + 1)],
                    h[:, MMCH * c:MMCH * (c + 1)],
                    w2s[:],
                    start=(c == 0), stop=(c == nch - 1),
                )

        def emit_dve(st, Sb):
            pk, nch, t = st["pk"], st["nch"], st["t"]
            ncol = NBITS * nch
            Y = yp.tile([128, 312], BF16, name="Y", tag="Y")
            nc.vector.tensor_tensor(Y[:, :ncol], pk[:, :ncol], trep[:, :ncol],
                                    ALU.is_gt)
            Z = zp.tile([128, 312], BF16, name="Z", tag="Z")
            nc.vector.tensor_tensor(Z[:, :ncol], Y[:, :ncol], wrep[:, :ncol],
                                    ALU.mult)
            if OB_OFF[t] == 0:
                Sb = so_p.tile([128, OUTW], F32, name="Sb", tag="S")
            z4 = Z[:, :ncol].rearrange("p (c h b) -> p c h b", c=nch, h=2, b=13)
            off = OB_OFF[t]
            nc.vector.tensor_reduce(Sb[:, off:off + 2 * nch], z4, AX.X, ALU.add)
            bi = OB_OF[t]
            if t == OBATCH[bi][-1]:
                used = OB_USED[bi]
                nc.sync.dma_start(outp[bi][:, :used], Sb[:, :used])
            return Sb

        hpre_cur = l1mm(0, bits=bits0)

        Sb = None
        prev = None
        for t in range(N_ITERS):
            na = SIZES[t]
            nch = na // MMCH

            # ACT: Gelu for iter t (waits L1(t))
            h = hp.tile([128, CHUNK], F32, name="h", tag="h")
            nc.scalar.activation(
                h[:, :na], hpre_cur[:, :na], AF.Gelu, bias=b1c, scale=1.0
            )

            # PE: L1 for iter t+1 FIRST (only dep: Gelu(t-1) freed its
            # hpre slot, so it starts right away and finishes early)
            if t + 1 < N_ITERS:
                hpre_cur = l1mm(t + 1)

            # PE + DVE: previous iteration's full transposed L2 and pack
            if prev is not None:
                prev["pk"] = pkp.tile([128, 312], F32, name="pk", tag="pk")
                emit_l2(prev, 0, prev["nch"])
                Sb = emit_dve(prev, Sb)
            cur = {"h": h, "nch": nch, "t": t}
            if t >= N_ITERS - 4:
                # Un-defer the last four iterations so the DVE backlog
                # (otherwise one full iteration deep, strict FIFO) drains
                # during the final Gelus and the last DMA starts earlier.
                # Each L1(t+1) is still queued ahead of the L2 batches, so
                # the trailing Gelus are not delayed.
                cur["pk"] = pkp.tile([128, 312], F32, name="pk", tag="pk")
                emit_l2(cur, 0, cur["nch"])
                Sb = emit_dve(cur, Sb)
                prev = None
            else:
                prev = cur

            # Input prefetch ~6 iterations ahead. Issued after l1mm(t+1) so
            # the recycled R slot's readers are all already in the program.
            prefetch(t + 6)

        if prev is not None:
            prev["pk"] = pkp.tile([128, 312], F32, name="pk", tag="pk")
            emit_l2(prev, 0, prev["nch"])
            Sb = emit_dve(prev, Sb)

    return nc


def make_const_inputs(W1, b1, W2, b2):
    import ml_dtypes

    w1 = np.ascontiguousarray(W1[0:32, :], dtype=np.float32)
    hi = w1.astype(ml_dtypes.bfloat16)
    mid = (w1 - hi.astype(np.float32)).astype(ml_dtypes.bfloat16)
    lo = (w1 - hi.astype(np.float32) - mid.astype(np.float32)).astype(
        ml_dtypes.bfloat16
    )
    w1b = np.zeros((128, 128), dtype=ml_dtypes.bfloat16)
    w1b[0:32] = hi
    w1b[32:64] = mid
    w1b[64:96] = lo

    thr = (0.5 - np.asarray(b2[:NBITS], dtype=np.float32))  # [26]
    trep = np.tile(thr, 12)[None, :].repeat(128, axis=0)    # [128, 312]

    wvec = np.zeros(312, dtype=np.float32)
    for c in range(12):
        for h in range(2):
            for i in range(13):
                wvec[26 * c + 13 * h + i] = float(1 << i)
    wrep = wvec[None, :].repeat(128, axis=0).astype(ml_dtypes.bfloat16)

    cst = np.zeros((128, C_TOT), dtype=np.float32)
    cst[:, C_W2:C_W2 + NBITS] = np.asarray(W2[:, :NBITS], dtype=np.float32)
    cst[:, C_TR:C_TR + 312] = trep
    cst[:, C_WR:C_WR + 156] = np.ascontiguousarray(wrep).view(np.float32)

    bootw = np.zeros((128, BT_TOT), dtype=np.float32)
    bootw[:, BT_W1:BT_W1 + 64] = np.ascontiguousarray(w1b).view(np.float32)
    bootw[:, BT_B1] = np.asarray(b1, dtype=np.float32)
    return {"cst": cst, "bootw": bootw}


def make_bit_planes(virtual_addr):
    """Per-core [96, PER] bf16 0/1 bit planes (3x replicated)."""
    import ml_dtypes

    va32 = np.asarray(virtual_addr).astype(np.uint32)
    out = []
    for c in range(va32.size // PER):
        seg = va32[c * PER:(c + 1) * PER]
        bits = np.unpackbits(
            seg.view(np.uint8).reshape(-1, 4), axis=-1, bitorder="little"
        )  # [PER, 32]
        u16 = (bits.T.astype(np.uint16) * 0x3F80)  # [32, PER] bf16 bit pattern
        full = np.concatenate([u16, u16, u16], axis=0)  # [96, PER]
        out.append(np.ascontiguousarray(full).view(ml_dtypes.bfloat16))
    return out


def combine_output(o):
    """[NOUT, 128, OUTW] f32 -> [PER] int64."""
    res = np.empty(PER, dtype=np.int64)
    for t in range(N_ITERS):
        nch = SIZES[t] // MMCH
        off = OB_OFF[t]
        s = o[OB_OF[t], :, off:off + 2 * nch]       # [128, 2*nch]
        lo = s[:, 0::2].astype(np.int64)            # [128, nch]
        hi = s[:, 1::2].astype(np.int64)
        phys = (lo + 8192 * hi).T.reshape(-1)       # (chunk, partition) order
        res[CSTART[t]:CSTART[t + 1]] = phys
    return res


_NC_CACHE = {}
TRACE = False
LAST_RES = None


def kernel(virtual_addr, W1, b1, W2, b2):
    global LAST_RES
    if "nc" not in _NC_CACHE:
        nc = build_nc()
        nc.finalize()
        _NC_CACHE["nc"] = nc
    nc = _NC_CACHE["nc"]

    consts = make_const_inputs(W1, b1, W2, b2)
    planes = make_bit_planes(virtual_addr)
    in_maps = []
    for c in range(NCORES):
        boot = consts["bootw"].copy()
        boot[0:96, BT_BITS:BT_BITS + 256] = (
            np.ascontiguousarray(planes[c][:, :512]).view(np.float32)
        )
        in_maps.append({"bp": planes[c], "cst": consts["cst"], "boot": boot})

    res = bass_utils.run_bass_kernel_spmd(
        nc, in_maps, list(range(NCORES)), trace=TRACE
    )
    LAST_RES = res

    outs = [combine_output(res.results[c]["outp"]) for c in range(NCORES)]
    return np.concatenate(outs)


# revision 16
# speedup vs baseline: 1.0043x; 1.0043x over previous
"""NeuralMMU Trainium2 kernel, v2 — transposed second layer.

Per core (131072 addrs), 87 iterations sized [512, 1024, 84x1536, 512]
(small first iterations so the first Gelu starts early; small final
iteration so the pipeline drain is short).

  1. Host sends bit planes as bf16 [96, 131072] (bit k of addr a at
     partition k, replicated 3x for the 3-way bf16 split of W1). A
     single `boot` DMA carries W1+b1 plus iteration-0's bits so one
     DMA chain gates the first L1/Gelu; input groups are single
     iterations during ramp-up, pairs afterwards, prefetched ~6
     iterations ahead through 4 rotating buffers.
  2. L1: bf16 matmuls k=96 (512-addr blocks): bits @ (W1hi;W1mid;W1lo)
     -> PSUM hpre [128, <=1536] (exact: bits are 0/1, f32 accumulate).
  3. ACT Gelu(+b1): PSUM -> SBUF h f32, one instr/iter. This is the
     modeled bottleneck (~0.83 ns/elem + ~185 ns init per instr,
     ~126.6 us busy); everything else hides behind it.
  4. L2 TRANSPOSED: per 128-addr chunk, matmul with the h chunk
     [128 hid, 128 addr] as the *stationary* operand and W2[:, :26] f32
     as the *moving* operand -> PSUM pk [128 addr, nch*26] f32, exact,
     all chunks in one PSUM bank (start on first chunk, stop on last).
     Model cost 26*4 cyc/chunk vs 512*4 cyc per 512 addrs when h is the
     moving side: ~4.7x less PE time for the heavy layer. (LDWEIGHTS
     per chunk is free in the cost model and pipelined on silicon.)
  5. DVE: is_gt vs replicated thresholds (0.5 - b2[k]) -> bf16 bits,
     multiply by replicated 2^i weights (i = bit index within the lo/hi
     13-bit half), 4D tensor_reduce -> [128, 2*nch] f32 (lo, hi).
  6. Output batched 16 iters per DMA + a singleton final batch; host
     combines lo + 8192*hi.

PSUM: hpre 2 bufs x 3 banks + pk 2 bufs x 1 bank = 8 banks exactly.
(CHUNK=1664 by sharing pk into hpre's 4th bank was tried and is ~44 us
SLOWER: the bank's zero-region start=True forces a Tile dependency of
the next L1 on pk's DVE reader, serializing ACT behind DVE.)
Cost model (graded metric) 135.4 us vs 299.4 us baseline; correctness
runs on real silicon via PJRT (1/1048576 borderline mismatch, same as
the baseline, rel err 0.0065).
"""

import numpy as np
from contextlib import ExitStack

import concourse.bass as bass
import concourse.mybir as mybir
import concourse.tile as tile
from concourse import bacc, bass_utils

B = 1_048_576
NCORES = 8
PER = B // NCORES            # 131072 addrs per core
BLK = 512                    # addrs per L1 matmul block
CHUNK = 1536                 # max addrs per iteration (3 PSUM banks)
MMCH = 128                   # addrs per transposed L2 matmul
NBITS = 26

# Alternating sizes: even iterations use the 4-bank PSUM tile (<=2048
# addrs), odd the 3-bank one (<=1536); pk single-buffered in bank 8.
# Under the deferred-L2 schedule the pk WAR has a full Gelu of slack.
SIZES = [512, 1024] + [2048, 1536] * 36 + [512]
assert sum(SIZES) == PER
N_ITERS = len(SIZES)         # 75
assert all(SIZES[t] <= (2048, 1536)[t % 2] for t in range(N_ITERS))
CSTART = [0]
for _s in SIZES:
    CSTART.append(CSTART[-1] + _s)

# Input DMA groups (lists of iterations): singles during ramp-up, pairs
# after. Iteration 0's bits ride in the boot tensor, not in a group.
GROUPS = ([[1], [2], [3], [4]] + [[i, i + 1] for i in range(5, 72, 2)]
          + [[73, 74]])
assert [t for g in GROUPS for t in g] == list(range(1, N_ITERS))
GRP_OF = {}
for _gi, _g in enumerate(GROUPS):
    for _t in _g:
        GRP_OF[_t] = _gi

# Output DMA batches: sixteen iterations each, then the tail alone so
# the final DMA after the last compute is tiny.
OBATCH = [list(range(r, min(r + 16, 74))) for r in range(0, 74, 16)] + [[74]]
OB_OF = {}
OB_OFF = {}
OB_USED = []
for _bi, _b in enumerate(OBATCH):
    used = 0
    for _t in _b:
        OB_OF[_t] = _bi
        OB_OFF[_t] = used
        used += 2 * (SIZES[_t] // MMCH)
    OB_USED.append(used)
NOUT = len(OBATCH)
OUTW = 28 * 16

F32 = mybir.dt.float32
BF16 = mybir.dt.bfloat16
AF = mybir.ActivationFunctionType
ALU = mybir.AluOpType
AX = mybir.AxisListType

# cst column layout (f32 columns); part A (w1b + b1) is DMA'd first so
# L1/Gelu can start before the larger part B arrives.
# boot tensor: w1b + b1 + iteration-0 bit planes, fetched in ONE DMA so a
# single 5-stage DMA chain gates the first L1/Gelu.
BT_W1 = 0         # [128, 64] f32 = [128, 128] bf16 3-way W1 split
BT_B1 = 64        # [128, 1] f32
BT_BITS = 65      # [96, 256] f32 = [96, 512] bf16 iteration-0 bits
BT_TOT = 65 + 256
# cst tensor (rest of the constants)
MAXCH = 16        # max transposed-L2 chunks per iteration
NREP = 26 * MAXCH  # 416 replicated threshold/weight columns
C_W2 = 0          # [128, 26] f32
C_TR = 26         # [128, 416] f32 thresholds replicated x16
C_WR = 442        # [128, 208] f32 = [128, 416] bf16 pack weights 2^i
C_TOT = 650


def build_nc() -> bass.Bass:
    nc = bacc.Bacc("TRN2")

    bp = nc.dram_tensor("bp", [96, PER], BF16, kind="ExternalInput")
    boot_d = nc.dram_tensor("boot", [128, BT_TOT], F32, kind="ExternalInput")
    cst_d = nc.dram_tensor("cst", [128, C_TOT], F32, kind="ExternalInput")
    outp = nc.dram_tensor("outp", [NOUT, 128, OUTW], F32, kind="ExternalOutput")

    with ExitStack() as ctx:
        tc = ctx.enter_context(tile.TileContext(nc))
        const = ctx.enter_context(tc.tile_pool(name="const", bufs=1))
        rpool = ctx.enter_context(tc.tile_pool(name="rp", bufs=4))
        hpre_p = ctx.enter_context(tc.tile_pool(name="hpre", bufs=1, space="PSUM"))
        hp = ctx.enter_context(tc.tile_pool(name="hp", bufs=3))
        pkp = ctx.enter_context(tc.tile_pool(name="pkp", bufs=1, space="PSUM"))
        yp = ctx.enter_context(tc.tile_pool(name="yp", bufs=2))
        zp = ctx.enter_context(tc.tile_pool(name="zp", bufs=2))
        so_p = ctx.enter_context(tc.tile_pool(name="so", bufs=2))

        boot = const.tile([128, BT_TOT], F32)
        nc.sync.dma_start(boot[:], boot_d[:])
        cst = const.tile([128, C_TOT], F32)

        w1b = boot[:, BT_W1:BT_W1 + 64].bitcast(BF16)    # [128, 128] bf16
        b1c = boot[:, BT_B1:BT_B1 + 1]
        bits0 = boot[0:96, BT_BITS:BT_BITS + 256].bitcast(BF16)  # [96, 512]
        w2s = cst[:, C_W2:C_W2 + NBITS]                  # [128, 26] f32
        trep = cst[:, C_TR:C_TR + NREP]                  # [128, 416] f32
        wrep = cst[:, C_WR:C_WR + NREP // 2].bitcast(BF16)  # [128, 416] bf16

        R = [None, None, None, None]
        next_group = 0

        def load_group(gi):
            g = GROUPS[gi]
            lo, hi = CSTART[g[0]], CSTART[g[-1] + 1]
            Rg = rpool.tile([96, 3584], BF16, name="Rg", tag="R")
            nc.sync.dma_start(Rg[:, : hi - lo], bp[:, lo:hi])
            R[gi % 4] = Rg

        def prefetch(upto_iter):
            nonlocal next_group
            while (next_group < len(GROUPS)
                   and GROUPS[next_group][0] <= upto_iter):
                load_group(next_group)
                next_group += 1

        def bits_of(t):
            gi = GRP_OF[t]
            off = CSTART[t] - CSTART[GROUPS[gi][0]]
            return R[gi % 4][0:96, off:off + SIZES[t]]

        ringA = hpre_p.tile([128, 2048], F32, name="ringA", tag="ringA")
        ringB = hpre_p.tile([128, 1536], F32, name="ringB", tag="ringB")
        rings = [ringA, ringB]

        def l1mm(t, bits=None):
            na = SIZES[t]
            hpre = rings[t % 2]
            if bits is None:
                bits = bits_of(t)
            for b in range(0, na, BLK):
                w = min(BLK, na - b)
                nc.tensor.matmul(
                    hpre[:, b:b + w],
                    w1b[0:96, :],
                    bits[:, b:b + w],
                    start=True, stop=True, tile_position=(0, 0),
                )
            return hpre[:, 0:na]

        # Startup: boot already issued; then bits for iterations 1-2, the
        # remaining constants, and iteration 3; in-loop prefetch takes over.
        prefetch(2)
        nc.sync.dma_start(cst[:], cst_d[:])
        prefetch(3)
        # Defer each iteration's entire transposed-L2 to the next
        # iteration's PE slot, and put L1(t+1) FIRST there: the in-order PE
        # queue then completes L1(t+1) ~880ns into Gelu(t)'s 1465ns window
        # instead of ~40ns after it (which cost a sem+decode stall on every
        # Gelu). h is triple-buffered so the deferred L2'(t-1) read never
        # collides with Gelu(t+1)'s output slot.
        def emit_l2(st, c0, c1):
            pk, h, nch = st["pk"], st["h"], st["nch"]
            for c in range(c0, min(c1, nch)):
                nc.tensor.matmul(
                    pk[:, NBITS * c:NBITS * (c + 1)],
                    h[:, MMCH * c:MMCH * (c + 1)],
                    w2s[:],
                    start=(c == 0), stop=(c == nch - 1),
                )

        def emit_dve(st, Sb):
            pk, nch, t = st["pk"], st["nch"], st["t"]
            ncol = NBITS * nch
            Y = yp.tile([128, NREP], BF16, name="Y", tag="Y")
            nc.vector.tensor_tensor(Y[:, :ncol], pk[:, :ncol], trep[:, :ncol],
                                    ALU.is_gt)
            Z = zp.tile([128, NREP], BF16, name="Z", tag="Z")
            nc.vector.tensor_tensor(Z[:, :ncol], Y[:, :ncol], wrep[:, :ncol],
                                    ALU.mult)
            if OB_OFF[t] == 0:
                Sb = so_p.tile([128, OUTW], F32, name="Sb", tag="S")
            z4 = Z[:, :ncol].rearrange("p (c h b) -> p c h b", c=nch, h=2, b=13)
            off = OB_OFF[t]
            nc.vector.tensor_reduce(Sb[:, off:off + 2 * nch], z4, AX.X, ALU.add)
            bi = OB_OF[t]
            if t == OBATCH[bi][-1]:
                used = OB_USED[bi]
                nc.sync.dma_start(outp[bi][:, :used], Sb[:, :used])
            return Sb

        hpre_cur = l1mm(0, bits=bits0)

        Sb = None
        prev = None
        for t in range(N_ITERS):
            na = SIZES[t]
            nch = na // MMCH

            # ACT: Gelu for iter t (waits L1(t))
            h = hp.tile([128, 2048], F32, name="h", tag="h")
            nc.scalar.activation(
                h[:, :na], hpre_cur, AF.Gelu, bias=b1c, scale=1.0
            )

            # PE: L1 for iter t+1 FIRST (only dep: Gelu(t-1) freed its
            # hpre slot, so it starts right away and finishes early)
            if t + 1 < N_ITERS:
                hpre_cur = l1mm(t + 1)

            # PE + DVE: previous iteration's full transposed L2 and pack
            if prev is not None:
                prev["pk"] = pkp.tile([128, NREP], F32, name="pk", tag="pk")
                emit_l2(prev, 0, prev["nch"])
                Sb = emit_dve(prev, Sb)
            cur = {"h": h, "nch": nch, "t": t}
            if t >= N_ITERS - 6:
                # Un-defer the last four iterations so the DVE backlog
                # (otherwise one full iteration deep, strict FIFO) drains
                # during the final Gelus and the last DMA starts earlier.
                # Each L1(t+1) is still queued ahead of the L2 batches, so
                # the trailing Gelus are not delayed.
                cur["pk"] = pkp.tile([128, NREP], F32, name="pk", tag="pk")
                emit_l2(cur, 0, cur["nch"])
                Sb = emit_dve(cur, Sb)
                prev = None
            else:
                prev = cur

            # Input prefetch ~6 iterations ahead. Issued after l1mm(t+1) so
            # the recycled R slot's readers are all already in the program.
            prefetch(t + 6)

        if prev is not None:
            prev["pk"] = pkp.tile([128, NREP], F32, name="pk", tag="pk")
            emit_l2(prev, 0, prev["nch"])
            Sb = emit_dve(prev, Sb)

    return nc


def make_const_inputs(W1, b1, W2, b2):
    import ml_dtypes

    w1 = np.ascontiguousarray(W1[0:32, :], dtype=np.float32)
    hi = w1.astype(ml_dtypes.bfloat16)
    mid = (w1 - hi.astype(np.float32)).astype(ml_dtypes.bfloat16)
    lo = (w1 - hi.astype(np.float32) - mid.astype(np.float32)).astype(
        ml_dtypes.bfloat16
    )
    w1b = np.zeros((128, 128), dtype=ml_dtypes.bfloat16)
    w1b[0:32] = hi
    w1b[32:64] = mid
    w1b[64:96] = lo

    thr = (0.5 - np.asarray(b2[:NBITS], dtype=np.float32))  # [26]
    trep = np.tile(thr, MAXCH)[None, :].repeat(128, axis=0)  # [128, 416]

    wvec = np.zeros(NREP, dtype=np.float32)
    for c in range(MAXCH):
        for h in range(2):
            for i in range(13):
                wvec[26 * c + 13 * h + i] = float(1 << i)
    wrep = wvec[None, :].repeat(128, axis=0).astype(ml_dtypes.bfloat16)

    cst = np.zeros((128, C_TOT), dtype=np.float32)
    cst[:, C_W2:C_W2 + NBITS] = np.asarray(W2[:, :NBITS], dtype=np.float32)
    cst[:, C_TR:C_TR + NREP] = trep
    cst[:, C_WR:C_WR + NREP // 2] = np.ascontiguousarray(wrep).view(np.float32)

    bootw = np.zeros((128, BT_TOT), dtype=np.float32)
    bootw[:, BT_W1:BT_W1 + 64] = np.ascontiguousarray(w1b).view(np.float32)
    bootw[:, BT_B1] = np.asarray(b1, dtype=np.float32)
    return {"cst": cst, "bootw": bootw}


def make_bit_planes(virtual_addr):
    """Per-core [96, PER] bf16 0/1 bit planes (3x replicated)."""
    import ml_dtypes

    va32 = np.asarray(virtual_addr).astype(np.uint32)
    out = []
    for c in range(va32.size // PER):
        seg = va32[c * PER:(c + 1) * PER]
        bits = np.unpackbits(
            seg.view(np.uint8).reshape(-1, 4), axis=-1, bitorder="little"
        )  # [PER, 32]
        u16 = (bits.T.astype(np.uint16) * 0x3F80)  # [32, PER] bf16 bit pattern
        full = np.concatenate([u16, u16, u16], axis=0)  # [96, PER]
        out.append(np.ascontiguousarray(full).view(ml_dtypes.bfloat16))
    return out


def combine_output(o):
    """[NOUT, 128, OUTW] f32 -> [PER] int64."""
    res = np.empty(PER, dtype=np.int64)
    for t in range(N_ITERS):
        nch = SIZES[t] // MMCH
        off = OB_OFF[t]
        s = o[OB_OF[t], :, off:off + 2 * nch]       # [128, 2*nch]
        lo = s[:, 0::2].astype(np.int64)            # [128, nch]
        hi = s[:, 1::2].astype(np.int64)
        phys = (lo + 8192 * hi).T.reshape(-1)       # (chunk, partition) order
        res[CSTART[t]:CSTART[t + 1]] = phys
    return res


_NC_CACHE = {}
TRACE = False
LAST_RES = None


def kernel(virtual_addr, W1, b1, W2, b2):
    global LAST_RES
    if "nc" not in _NC_CACHE:
        nc = build_nc()
        nc.finalize()
        _NC_CACHE["nc"] = nc
    nc = _NC_CACHE["nc"]

    consts = make_const_inputs(W1, b1, W2, b2)
    planes = make_bit_planes(virtual_addr)
    in_maps = []
    for c in range(NCORES):
        boot = consts["bootw"].copy()
        boot[0:96, BT_BITS:BT_BITS + 256] = (
            np.ascontiguousarray(planes[c][:, :512]).view(np.float32)
        )
        in_maps.append({"bp": planes[c], "cst": consts["cst"], "boot": boot})

    res = bass_utils.run_bass_kernel_spmd(
        nc, in_maps, list(range(NCORES)), trace=TRACE
    )
    LAST_RES = res

    outs = [combine_output(res.results[c]["outp"]) for c in range(NCORES)]
    return np.concatenate(outs)


# revision 18
# speedup vs baseline: 1.0079x; 1.0036x over previous
"""NeuralMMU Trainium2 kernel, v2 — transposed second layer.

Per core (131072 addrs), 75 iterations sized [512, 1024, 36x(2048,1536),
512]: small first iterations so the first Gelu starts early, a small
final iteration so the pipeline drain is short, and alternating
2048/1536 in between (asymmetric PSUM tiles ringA 4 banks / ringB 3
banks + single-buffered pk in bank 8 — legal only because the deferred
L2 gives the pk WAR a full Gelu of slack).

  1. Host sends bit planes as bf16 [96, 131072] (bit k of addr a at
     partition k, replicated 3x for the 3-way bf16 split of W1). A
     single `boot` DMA carries W1+b1 plus iteration-0's bits so one
     DMA chain gates the first L1/Gelu; input groups are single
     iterations during ramp-up, pairs afterwards, prefetched ~6
     iterations ahead through 4 rotating buffers.
  2. L1: bf16 matmuls k=96 (512-addr blocks): bits @ (W1hi;W1mid;W1lo)
     -> PSUM hpre [128, <=1536] (exact: bits are 0/1, f32 accumulate).
  3. ACT Gelu(+b1): PSUM -> SBUF h f32, one instr/iter. This is the
     modeled bottleneck (~0.83 ns/elem + ~185 ns init per instr,
     ~126.6 us busy); everything else hides behind it.
  4. L2 TRANSPOSED: per 128-addr chunk, matmul with the h chunk
     [128 hid, 128 addr] as the *stationary* operand and W2[:, :26] f32
     as the *moving* operand -> PSUM pk [128 addr, nch*26] f32, exact,
     all chunks in one PSUM bank (start on first chunk, stop on last).
     Model cost 26*4 cyc/chunk vs 512*4 cyc per 512 addrs when h is the
     moving side: ~4.7x less PE time for the heavy layer. (LDWEIGHTS
     per chunk is free in the cost model and pipelined on silicon.)
  5. DVE: is_gt vs replicated thresholds (0.5 - b2[k]) -> bf16 bits,
     multiply by replicated 2^i weights (i = bit index within the lo/hi
     13-bit half), 4D tensor_reduce -> [128, 2*nch] f32 (lo, hi).
  6. Output batched 16 iters per DMA + a singleton final batch; host
     combines lo + 8192*hi.

PSUM: hpre 2 bufs x 3 banks + pk 2 bufs x 1 bank = 8 banks exactly.
(CHUNK=1664 by sharing pk into hpre's 4th bank was tried and is ~44 us
SLOWER: the bank's zero-region start=True forces a Tile dependency of
the next L1 on pk's DVE reader, serializing ACT behind DVE.)
Cost model (graded metric) 134.9 us vs 299.4 us baseline; correctness
runs on real silicon via PJRT (1/1048576 borderline mismatch, same as
the baseline, rel err 0.0065).
"""

import numpy as np
from contextlib import ExitStack

import concourse.bass as bass
import concourse.mybir as mybir
import concourse.tile as tile
from concourse import bacc, bass_utils

B = 1_048_576
NCORES = 8
PER = B // NCORES            # 131072 addrs per core
BLK = 512                    # addrs per L1 matmul block
CHUNK = 1536                 # max addrs per iteration (3 PSUM banks)
MMCH = 128                   # addrs per transposed L2 matmul
NBITS = 26

# Alternating sizes: even iterations use the 4-bank PSUM tile (<=2048
# addrs), odd the 3-bank one (<=1536); pk single-buffered in bank 8.
# Under the deferred-L2 schedule the pk WAR has a full Gelu of slack.
SIZES = [512, 1536] + [2048, 1536] * 35 + [1536, 1024, 1024]
assert sum(SIZES) == PER
N_ITERS = len(SIZES)
assert all(SIZES[t] <= (2048, 1536)[t % 2] for t in range(N_ITERS))
CSTART = [0]
for _s in SIZES:
    CSTART.append(CSTART[-1] + _s)

# Input DMA groups (lists of iterations): singles during ramp-up, pairs
# after. Iteration 0's bits ride in the boot tensor, not in a group.
GROUPS = ([[1], [2], [3], [4]] + [[i, i + 1] for i in range(5, 72, 2)]
          + [[73, 74]])
assert [t for g in GROUPS for t in g] == list(range(1, N_ITERS))
GRP_OF = {}
for _gi, _g in enumerate(GROUPS):
    for _t in _g:
        GRP_OF[_t] = _gi

# Output DMA batches: sixteen iterations each, then the tail alone so
# the final DMA after the last compute is tiny.
OBATCH = [list(range(r, min(r + 16, 74))) for r in range(0, 74, 16)] + [[74]]
OB_OF = {}
OB_OFF = {}
OB_USED = []
for _bi, _b in enumerate(OBATCH):
    used = 0
    for _t in _b:
        OB_OF[_t] = _bi
        OB_OFF[_t] = used
        used += 2 * (SIZES[_t] // MMCH)
    OB_USED.append(used)
NOUT = len(OBATCH)
OUTW = 28 * 16

F32 = mybir.dt.float32
BF16 = mybir.dt.bfloat16
AF = mybir.ActivationFunctionType
ALU = mybir.AluOpType
AX = mybir.AxisListType

# cst column layout (f32 columns); part A (w1b + b1) is DMA'd first so
# L1/Gelu can start before the larger part B arrives.
# boot tensor: w1b + b1 + iteration-0 bit planes, fetched in ONE DMA so a
# single 5-stage DMA chain gates the first L1/Gelu.
BT_W1 = 0         # [128, 64] f32 = [128, 128] bf16 3-way W1 split
BT_B1 = 64        # [128, 1] f32
BT_BITS = 65      # [96, 256] f32 = [96, 512] bf16 iteration-0 bits
BT_TOT = 65 + 256
# cst tensor (rest of the constants)
MAXCH = 16        # max transposed-L2 chunks per iteration
NREP = 26 * MAXCH  # 416 replicated threshold/weight columns
C_W2 = 0          # [128, 26] f32
C_TR = 26         # [128, 416] f32 thresholds replicated x16
C_WR = 442        # [128, 208] f32 = [128, 416] bf16 pack weights 2^i
C_TOT = 650


def build_nc() -> bass.Bass:
    nc = bacc.Bacc("TRN2")

    bp = nc.dram_tensor("bp", [96, PER], BF16, kind="ExternalInput")
    boot_d = nc.dram_tensor("boot", [128, BT_TOT], F32, kind="ExternalInput")
    cst_d = nc.dram_tensor("cst", [128, C_TOT], F32, kind="ExternalInput")
    outp = nc.dram_tensor("outp", [NOUT, 128, OUTW], F32, kind="ExternalOutput")

    with ExitStack() as ctx:
        tc = ctx.enter_context(tile.TileContext(nc))
        const = ctx.enter_context(tc.tile_pool(name="const", bufs=1))
        rpool = ctx.enter_context(tc.tile_pool(name="rp", bufs=4))
        hpre_p = ctx.enter_context(tc.tile_pool(name="hpre", bufs=1, space="PSUM"))
        hp = ctx.enter_context(tc.tile_pool(name="hp", bufs=3))
        pkp = ctx.enter_context(tc.tile_pool(name="pkp", bufs=1, space="PSUM"))
        yp = ctx.enter_context(tc.tile_pool(name="yp", bufs=2))
        zp = ctx.enter_context(tc.tile_pool(name="zp", bufs=2))
        so_p = ctx.enter_context(tc.tile_pool(name="so", bufs=2))

        boot = const.tile([128, BT_TOT], F32)
        nc.sync.dma_start(boot[:], boot_d[:])
        cst = const.tile([128, C_TOT], F32)

        w1b = boot[:, BT_W1:BT_W1 + 64].bitcast(BF16)    # [128, 128] bf16
        b1c = boot[:, BT_B1:BT_B1 + 1]
        bits0 = boot[0:96, BT_BITS:BT_BITS + 256].bitcast(BF16)  # [96, 512]
        w2s = cst[:, C_W2:C_W2 + NBITS]                  # [128, 26] f32
        trep = cst[:, C_TR:C_TR + NREP]                  # [128, 416] f32
        wrep = cst[:, C_WR:C_WR + NREP // 2].bitcast(BF16)  # [128, 416] bf16

        R = [None, None, None, None]
        next_group = 0

        def load_group(gi):
            g = GROUPS[gi]
            lo, hi = CSTART[g[0]], CSTART[g[-1] + 1]
            Rg = rpool.tile([96, 3584], BF16, name="Rg", tag="R")
            nc.sync.dma_start(Rg[:, : hi - lo], bp[:, lo:hi])
            R[gi % 4] = Rg

        def prefetch(upto_iter):
            nonlocal next_group
            while (next_group < len(GROUPS)
                   and GROUPS[next_group][0] <= upto_iter):
                load_group(next_group)
                next_group += 1

        def bits_of(t):
            gi = GRP_OF[t]
            off = CSTART[t] - CSTART[GROUPS[gi][0]]
            return R[gi % 4][0:96, off:off + SIZES[t]]

        ringA = hpre_p.tile([128, 2048], F32, name="ringA", tag="ringA")
        ringB = hpre_p.tile([128, 1536], F32, name="ringB", tag="ringB")
        rings = [ringA, ringB]

        def l1mm(t, bits=None):
            na = SIZES[t]
            hpre = rings[t % 2]
            if bits is None:
                bits = bits_of(t)
            for b in range(0, na, BLK):
                w = min(BLK, na - b)
                nc.tensor.matmul(
                    hpre[:, b:b + w],
                    w1b[0:96, :],
                    bits[:, b:b + w],
                    start=True, stop=True, tile_position=(0, 0),
                )
            return hpre[:, 0:na]

        # Startup: boot already issued; then bits for iterations 1-2, the
        # remaining constants, and iteration 3; in-loop prefetch takes over.
        prefetch(2)
        nc.sync.dma_start(cst[:], cst_d[:])
        prefetch(3)
        # Defer each iteration's entire transposed-L2 to the next
        # iteration's PE slot, and put L1(t+1) FIRST there: the in-order PE
        # queue then completes L1(t+1) ~880ns into Gelu(t)'s 1465ns window
        # instead of ~40ns after it (which cost a sem+decode stall on every
        # Gelu). h is triple-buffered so the deferred L2'(t-1) read never
        # collides with Gelu(t+1)'s output slot.
        def emit_l2(st, c0, c1):
            pk, h, nch = st["pk"], st["h"], st["nch"]
            for c in range(c0, min(c1, nch)):
                nc.tensor.matmul(
                    pk[:, NBITS * c:NBITS * (c + 1)],
                    h[:, MMCH * c:MMCH * (c + 1)],
                    w2s[:],
                    start=(c == 0), stop=(c == nch - 1),
                )

        def emit_dve(st, Sb):
            pk, nch, t = st["pk"], st["nch"], st["t"]
            ncol = NBITS * nch
            Y = yp.tile([128, NREP], BF16, name="Y", tag="Y")
            nc.vector.tensor_tensor(Y[:, :ncol], pk[:, :ncol], trep[:, :ncol],
                                    ALU.is_gt)
            Z = zp.tile([128, NREP], BF16, name="Z", tag="Z")
            nc.vector.tensor_tensor(Z[:, :ncol], Y[:, :ncol], wrep[:, :ncol],
                                    ALU.mult)
            if OB_OFF[t] == 0:
                Sb = so_p.tile([128, OUTW], F32, name="Sb", tag="S")
            z4 = Z[:, :ncol].rearrange("p (c h b) -> p c h b", c=nch, h=2, b=13)
            off = OB_OFF[t]
            nc.vector.tensor_reduce(Sb[:, off:off + 2 * nch], z4, AX.X, ALU.add)
            bi = OB_OF[t]
            if t == OBATCH[bi][-1]:
                used = OB_USED[bi]
                nc.sync.dma_start(outp[bi][:, :used], Sb[:, :used])
            return Sb

        hpre_cur = l1mm(0, bits=bits0)

        Sb = None
        prev = None
        for t in range(N_ITERS):
            na = SIZES[t]
            nch = na // MMCH

            # ACT: Gelu for iter t (waits L1(t))
            h = hp.tile([128, 2048], F32, name="h", tag="h")
            nc.scalar.activation(
                h[:, :na], hpre_cur, AF.Gelu, bias=b1c, scale=1.0
            )

            # PE: L1 for iter t+1 FIRST (only dep: Gelu(t-1) freed its
            # hpre slot, so it starts right away and finishes early)
            if t + 1 < N_ITERS:
                hpre_cur = l1mm(t + 1)

            # PE + DVE: previous iteration's full transposed L2 and pack
            if prev is not None:
                prev["pk"] = pkp.tile([128, NREP], F32, name="pk", tag="pk")
                emit_l2(prev, 0, prev["nch"])
                Sb = emit_dve(prev, Sb)
            cur = {"h": h, "nch": nch, "t": t}
            if t >= N_ITERS - 6:
                # Un-defer the last four iterations so the DVE backlog
                # (otherwise one full iteration deep, strict FIFO) drains
                # during the final Gelus and the last DMA starts earlier.
                # Each L1(t+1) is still queued ahead of the L2 batches, so
                # the trailing Gelus are not delayed.
                cur["pk"] = pkp.tile([128, NREP], F32, name="pk", tag="pk")
                emit_l2(cur, 0, cur["nch"])
                Sb = emit_dve(cur, Sb)
                prev = None
            else:
                prev = cur

            # Input prefetch ~6 iterations ahead. Issued after l1mm(t+1) so
            # the recycled R slot's readers are all already in the program.
            prefetch(t + 6)

        if prev is not None:
            prev["pk"] = pkp.tile([128, NREP], F32, name="pk", tag="pk")
            emit_l2(prev, 0, prev["nch"])
            Sb = emit_dve(prev, Sb)

    return nc


def make_const_inputs(W1, b1, W2, b2):
    import ml_dtypes

    w1 = np.ascontiguousarray(W1[0:32, :], dtype=np.float32)
    hi = w1.astype(ml_dtypes.bfloat16)
    mid = (w1 - hi.astype(np.float32)).astype(ml_dtypes.bfloat16)
    lo = (w1 - hi.astype(np.float32) - mid.astype(np.float32)).astype(
        ml_dtypes.bfloat16
    )
    w1b = np.zeros((128, 128), dtype=ml_dtypes.bfloat16)
    w1b[0:32] = hi
    w1b[32:64] = mid
    w1b[64:96] = lo

    thr = (0.5 - np.asarray(b2[:NBITS], dtype=np.float32))  # [26]
    trep = np.tile(thr, MAXCH)[None, :].repeat(128, axis=0)  # [128, 416]

    wvec = np.zeros(NREP, dtype=np.float32)
    for c in range(MAXCH):
        for h in range(2):
            for i in range(13):
                wvec[26 * c + 13 * h + i] = float(1 << i)
    wrep = wvec[None, :].repeat(128, axis=0).astype(ml_dtypes.bfloat16)

    cst = np.zeros((128, C_TOT), dtype=np.float32)
    cst[:, C_W2:C_W2 + NBITS] = np.asarray(W2[:, :NBITS], dtype=np.float32)
    cst[:, C_TR:C_TR + NREP] = trep
    cst[:, C_WR:C_WR + NREP // 2] = np.ascontiguousarray(wrep).view(np.float32)

    bootw = np.zeros((128, BT_TOT), dtype=np.float32)
    bootw[:, BT_W1:BT_W1 + 64] = np.ascontiguousarray(w1b).view(np.float32)
    bootw[:, BT_B1] = np.asarray(b1, dtype=np.float32)
    return {"cst": cst, "bootw": bootw}


def make_bit_planes(virtual_addr):
    """Per-core [96, PER] bf16 0/1 bit planes (3x replicated)."""
    import ml_dtypes

    va32 = np.asarray(virtual_addr).astype(np.uint32)
    out = []
    for c in range(va32.size // PER):
        seg = va32[c * PER:(c + 1) * PER]
        bits = np.unpackbits(
            seg.view(np.uint8).reshape(-1, 4), axis=-1, bitorder="little"
        )  # [PER, 32]
        u16 = (bits.T.astype(np.uint16) * 0x3F80)  # [32, PER] bf16 bit pattern
        full = np.concatenate([u16, u16, u16], axis=0)  # [96, PER]
        out.append(np.ascontiguousarray(full).view(ml_dtypes.bfloat16))
    return out


def combine_output(o):
    """[NOUT, 128, OUTW] f32 -> [PER] int64."""
    res = np.empty(PER, dtype=np.int64)
    for t in range(N_ITERS):
        nch = SIZES[t] // MMCH
        off = OB_OFF[t]
        s = o[OB_OF[t], :, off:off + 2 * nch]       # [128, 2*nch]
        lo = s[:, 0::2].astype(np.int64)            # [128, nch]
        hi = s[:, 1::2].astype(np.int64)
        phys = (lo + 8192 * hi).T.reshape(-1)       # (chunk, partition) order
        res[CSTART[t]:CSTART[t + 1]] = phys
    return res


_NC_CACHE = {}
TRACE = False
LAST_RES = None


def kernel(virtual_addr, W1, b1, W2, b2):
    global LAST_RES
    if "nc" not in _NC_CACHE:
        nc = build_nc()
        nc.finalize()
        _NC_CACHE["nc"] = nc
    nc = _NC_CACHE["nc"]

    consts = make_const_inputs(W1, b1, W2, b2)
    planes = make_bit_planes(virtual_addr)
    in_maps = []
    for c in range(NCORES):
        boot = consts["bootw"].copy()
        boot[0:96, BT_BITS:BT_BITS + 256] = (
            np.ascontiguousarray(planes[c][:, :512]).view(np.float32)
        )
        in_maps.append({"bp": planes[c], "cst": consts["cst"], "boot": boot})

    res = bass_utils.run_bass_kernel_spmd(
        nc, in_maps, list(range(NCORES)), trace=TRACE
    )
    LAST_RES = res

    outs = [combine_output(res.results[c]["outp"]) for c in range(NCORES)]
    return np.concatenate(outs)


# revision 19
# speedup vs baseline: 1.0111x; 1.0032x over previous
"""NeuralMMU Trainium2 kernel, v2 — transposed second layer.

Per core (131072 addrs), 75 iterations sized [512, 1024, 36x(2048,1536),
512]: small first iterations so the first Gelu starts early, a small
final iteration so the pipeline drain is short, and alternating
2048/1536 in between (asymmetric PSUM tiles ringA 4 banks / ringB 3
banks + single-buffered pk in bank 8 — legal only because the deferred
L2 gives the pk WAR a full Gelu of slack).

  1. Host sends bit planes as bf16 [96, 131072] (bit k of addr a at
     partition k, replicated 3x for the 3-way bf16 split of W1). A
     single `boot` DMA carries W1+b1 plus iteration-0's bits so one
     DMA chain gates the first L1/Gelu; input groups are single
     iterations during ramp-up, pairs afterwards, prefetched ~6
     iterations ahead through 4 rotating buffers.
  2. L1: bf16 matmuls k=96 (512-addr blocks): bits @ (W1hi;W1mid;W1lo)
     -> PSUM hpre [128, <=1536] (exact: bits are 0/1, f32 accumulate).
  3. ACT Gelu(+b1): PSUM -> SBUF h f32, one instr/iter. This is the
     modeled bottleneck (~0.83 ns/elem + ~185 ns init per instr,
     ~126.6 us busy); everything else hides behind it.
  4. L2 TRANSPOSED: per 128-addr chunk, matmul with the h chunk
     [128 hid, 128 addr] as the *stationary* operand and W2[:, :26] f32
     as the *moving* operand -> PSUM pk [128 addr, nch*26] f32, exact,
     all chunks in one PSUM bank (start on first chunk, stop on last).
     Model cost 26*4 cyc/chunk vs 512*4 cyc per 512 addrs when h is the
     moving side: ~4.7x less PE time for the heavy layer. (LDWEIGHTS
     per chunk is free in the cost model and pipelined on silicon.)
  5. DVE: is_gt vs replicated thresholds (0.5 - b2[k]) -> bf16 bits,
     multiply by replicated 2^i weights (i = bit index within the lo/hi
     13-bit half), 4D tensor_reduce -> [128, 2*nch] f32 (lo, hi).
  6. Output batched 16 iters per DMA + a singleton final batch; host
     combines lo + 8192*hi.

PSUM: hpre 2 bufs x 3 banks + pk 2 bufs x 1 bank = 8 banks exactly.
(CHUNK=1664 by sharing pk into hpre's 4th bank was tried and is ~44 us
SLOWER: the bank's zero-region start=True forces a Tile dependency of
the next L1 on pk's DVE reader, serializing ACT behind DVE.)
Cost model (graded metric) 134.9 us vs 299.4 us baseline; correctness
runs on real silicon via PJRT (1/1048576 borderline mismatch, same as
the baseline, rel err 0.0065).
"""

import numpy as np
from contextlib import ExitStack

import concourse.bass as bass
import concourse.mybir as mybir
import concourse.tile as tile
from concourse import bacc, bass_utils

B = 1_048_576
NCORES = 8
PER = B // NCORES            # 131072 addrs per core
BLK = 512                    # addrs per L1 matmul block
CHUNK = 1536                 # max addrs per iteration (3 PSUM banks)
MMCH = 128                   # addrs per transposed L2 matmul
NBITS = 26

# Alternating sizes: even iterations use the 4-bank PSUM tile (<=2048
# addrs), odd the 3-bank one (<=1536); pk single-buffered in bank 8.
# Under the deferred-L2 schedule the pk WAR has a full Gelu of slack.
SIZES = [512, 1536] + [2048, 1536] * 34 + [1536, 1536, 1536, 1536, 1024]
assert sum(SIZES) == PER
N_ITERS = len(SIZES)
assert all(SIZES[t] <= (2048, 1536)[t % 2] for t in range(N_ITERS))
CSTART = [0]
for _s in SIZES:
    CSTART.append(CSTART[-1] + _s)

# Input DMA groups (lists of iterations): singles during ramp-up, pairs
# after. Iteration 0's bits ride in the boot tensor, not in a group.
GROUPS = ([[1], [2], [3], [4]] + [[i, i + 1] for i in range(5, 72, 2)]
          + [[73, 74]])
assert [t for g in GROUPS for t in g] == list(range(1, N_ITERS))
GRP_OF = {}
for _gi, _g in enumerate(GROUPS):
    for _t in _g:
        GRP_OF[_t] = _gi

# Output DMA batches: sixteen iterations each, then the tail alone so
# the final DMA after the last compute is tiny.
OBATCH = [list(range(r, min(r + 16, 74))) for r in range(0, 74, 16)] + [[74]]
OB_OF = {}
OB_OFF = {}
OB_USED = []
for _bi, _b in enumerate(OBATCH):
    used = 0
    for _t in _b:
        OB_OF[_t] = _bi
        OB_OFF[_t] = used
        used += 2 * (SIZES[_t] // MMCH)
    OB_USED.append(used)
NOUT = len(OBATCH)
OUTW = 28 * 16

F32 = mybir.dt.float32
BF16 = mybir.dt.bfloat16
AF = mybir.ActivationFunctionType
ALU = mybir.AluOpType
AX = mybir.AxisListType

# cst column layout (f32 columns); part A (w1b + b1) is DMA'd first so
# L1/Gelu can start before the larger part B arrives.
# boot tensor: w1b + b1 + iteration-0 bit planes, fetched in ONE DMA so a
# single 5-stage DMA chain gates the first L1/Gelu.
BT_W1 = 0         # [128, 64] f32 = [128, 128] bf16 3-way W1 split
BT_B1 = 64        # [128, 1] f32
BT_BITS = 65      # [96, 256] f32 = [96, 512] bf16 iteration-0 bits
BT_TOT = 65 + 256
# cst tensor (rest of the constants)
MAXCH = 16        # max transposed-L2 chunks per iteration
NREP = 26 * MAXCH  # 416 replicated threshold/weight columns
C_W2 = 0          # [128, 26] f32
C_TR = 26         # [128, 416] f32 thresholds replicated x16
C_WR = 442        # [128, 208] f32 = [128, 416] bf16 pack weights 2^i
C_TOT = 650


def build_nc() -> bass.Bass:
    nc = bacc.Bacc("TRN2")

    bp = nc.dram_tensor("bp", [96, PER], BF16, kind="ExternalInput")
    boot_d = nc.dram_tensor("boot", [128, BT_TOT], F32, kind="ExternalInput")
    cst_d = nc.dram_tensor("cst", [128, C_TOT], F32, kind="ExternalInput")
    outp = nc.dram_tensor("outp", [NOUT, 128, OUTW], F32, kind="ExternalOutput")

    with ExitStack() as ctx:
        tc = ctx.enter_context(tile.TileContext(nc))
        const = ctx.enter_context(tc.tile_pool(name="const", bufs=1))
        rpool = ctx.enter_context(tc.tile_pool(name="rp", bufs=4))
        hpre_p = ctx.enter_context(tc.tile_pool(name="hpre", bufs=1, space="PSUM"))
        hp = ctx.enter_context(tc.tile_pool(name="hp", bufs=3))
        pkp = ctx.enter_context(tc.tile_pool(name="pkp", bufs=1, space="PSUM"))
        yp = ctx.enter_context(tc.tile_pool(name="yp", bufs=2))
        zp = ctx.enter_context(tc.tile_pool(name="zp", bufs=2))
        so_p = ctx.enter_context(tc.tile_pool(name="so", bufs=2))

        boot = const.tile([128, BT_TOT], F32)
        nc.sync.dma_start(boot[:], boot_d[:])
        cst = const.tile([128, C_TOT], F32)

        w1b = boot[:, BT_W1:BT_W1 + 64].bitcast(BF16)    # [128, 128] bf16
        b1c = boot[:, BT_B1:BT_B1 + 1]
        bits0 = boot[0:96, BT_BITS:BT_BITS + 256].bitcast(BF16)  # [96, 512]
        w2s = cst[:, C_W2:C_W2 + NBITS]                  # [128, 26] f32
        trep = cst[:, C_TR:C_TR + NREP]                  # [128, 416] f32
        wrep = cst[:, C_WR:C_WR + NREP // 2].bitcast(BF16)  # [128, 416] bf16

        R = [None, None, None, None]
        next_group = 0

        def load_group(gi):
            g = GROUPS[gi]
            lo, hi = CSTART[g[0]], CSTART[g[-1] + 1]
            Rg = rpool.tile([96, 3584], BF16, name="Rg", tag="R")
            nc.sync.dma_start(Rg[:, : hi - lo], bp[:, lo:hi])
            R[gi % 4] = Rg

        def prefetch(upto_iter):
            nonlocal next_group
            while (next_group < len(GROUPS)
                   and GROUPS[next_group][0] <= upto_iter):
                load_group(next_group)
                next_group += 1

        def bits_of(t):
            gi = GRP_OF[t]
            off = CSTART[t] - CSTART[GROUPS[gi][0]]
            return R[gi % 4][0:96, off:off + SIZES[t]]

        ringA = hpre_p.tile([128, 2048], F32, name="ringA", tag="ringA")
        ringB = hpre_p.tile([128, 1536], F32, name="ringB", tag="ringB")
        rings = [ringA, ringB]

        def l1mm(t, bits=None):
            na = SIZES[t]
            hpre = rings[t % 2]
            if bits is None:
                bits = bits_of(t)
            for b in range(0, na, BLK):
                w = min(BLK, na - b)
                nc.tensor.matmul(
                    hpre[:, b:b + w],
                    w1b[0:96, :],
                    bits[:, b:b + w],
                    start=True, stop=True, tile_position=(0, 0),
                )
            return hpre[:, 0:na]

        # Startup: boot already issued; then bits for iterations 1-2, the
        # remaining constants, and iteration 3; in-loop prefetch takes over.
        prefetch(2)
        nc.sync.dma_start(cst[:], cst_d[:])
        prefetch(3)
        # Defer each iteration's entire transposed-L2 to the next
        # iteration's PE slot, and put L1(t+1) FIRST there: the in-order PE
        # queue then completes L1(t+1) ~880ns into Gelu(t)'s 1465ns window
        # instead of ~40ns after it (which cost a sem+decode stall on every
        # Gelu). h is triple-buffered so the deferred L2'(t-1) read never
        # collides with Gelu(t+1)'s output slot.
        def emit_l2(st, c0, c1):
            pk, h, nch = st["pk"], st["h"], st["nch"]
            for c in range(c0, min(c1, nch)):
                nc.tensor.matmul(
                    pk[:, NBITS * c:NBITS * (c + 1)],
                    h[:, MMCH * c:MMCH * (c + 1)],
                    w2s[:],
                    start=(c == 0), stop=(c == nch - 1),
                )

        def emit_dve(st, Sb):
            pk, nch, t = st["pk"], st["nch"], st["t"]
            ncol = NBITS * nch
            Y = yp.tile([128, NREP], BF16, name="Y", tag="Y")
            nc.vector.tensor_tensor(Y[:, :ncol], pk[:, :ncol], trep[:, :ncol],
                                    ALU.is_gt)
            Z = zp.tile([128, NREP], BF16, name="Z", tag="Z")
            nc.vector.tensor_tensor(Z[:, :ncol], Y[:, :ncol], wrep[:, :ncol],
                                    ALU.mult)
            if OB_OFF[t] == 0:
                Sb = so_p.tile([128, OUTW], F32, name="Sb", tag="S")
            z4 = Z[:, :ncol].rearrange("p (c h b) -> p c h b", c=nch, h=2, b=13)
            off = OB_OFF[t]
            nc.vector.tensor_reduce(Sb[:, off:off + 2 * nch], z4, AX.X, ALU.add)
            bi = OB_OF[t]
            if t == OBATCH[bi][-1]:
                used = OB_USED[bi]
                nc.sync.dma_start(outp[bi][:, :used], Sb[:, :used])
            return Sb

        hpre_cur = l1mm(0, bits=bits0)

        Sb = None
        prev = None
        for t in range(N_ITERS):
            na = SIZES[t]
            nch = na // MMCH

            # ACT: Gelu for iter t (waits L1(t))
            h = hp.tile([128, 2048], F32, name="h", tag="h")
            nc.scalar.activation(
                h[:, :na], hpre_cur, AF.Gelu, bias=b1c, scale=1.0
            )

            # PE: L1 for iter t+1 FIRST (only dep: Gelu(t-1) freed its
            # hpre slot, so it starts right away and finishes early)
            if t + 1 < N_ITERS:
                hpre_cur = l1mm(t + 1)

            # PE + DVE: previous iteration's full transposed L2 and pack
            if prev is not None:
                prev["pk"] = pkp.tile([128, NREP], F32, name="pk", tag="pk")
                emit_l2(prev, 0, prev["nch"])
                Sb = emit_dve(prev, Sb)
            cur = {"h": h, "nch": nch, "t": t}
            if t >= N_ITERS - 6:
                # Un-defer the last four iterations so the DVE backlog
                # (otherwise one full iteration deep, strict FIFO) drains
                # during the final Gelus and the last DMA starts earlier.
                # Each L1(t+1) is still queued ahead of the L2 batches, so
                # the trailing Gelus are not delayed.
                cur["pk"] = pkp.tile([128, NREP], F32, name="pk", tag="pk")
                emit_l2(cur, 0, cur["nch"])
                Sb = emit_dve(cur, Sb)
                prev = None
            else:
                prev = cur

            # Input prefetch ~6 iterations ahead. Issued after l1mm(t+1) so
            # the recycled R slot's readers are all already in the program.
            prefetch(t + 6)

        if prev is not None:
            prev["pk"] = pkp.tile([128, NREP], F32, name="pk", tag="pk")
            emit_l2(prev, 0, prev["nch"])
            Sb = emit_dve(prev, Sb)

    return nc


def make_const_inputs(W1, b1, W2, b2):
    import ml_dtypes

    w1 = np.ascontiguousarray(W1[0:32, :], dtype=np.float32)
    hi = w1.astype(ml_dtypes.bfloat16)
    mid = (w1 - hi.astype(np.float32)).astype(ml_dtypes.bfloat16)
    lo = (w1 - hi.astype(np.float32) - mid.astype(np.float32)).astype(
        ml_dtypes.bfloat16
    )
    w1b = np.zeros((128, 128), dtype=ml_dtypes.bfloat16)
    w1b[0:32] = hi
    w1b[32:64] = mid
    w1b[64:96] = lo

    thr = (0.5 - np.asarray(b2[:NBITS], dtype=np.float32))  # [26]
    trep = np.tile(thr, MAXCH)[None, :].repeat(128, axis=0)  # [128, 416]

    wvec = np.zeros(NREP, dtype=np.float32)
    for c in range(MAXCH):
        for h in range(2):
            for i in range(13):
                wvec[26 * c + 13 * h + i] = float(1 << i)
    wrep = wvec[None, :].repeat(128, axis=0).astype(ml_dtypes.bfloat16)

    cst = np.zeros((128, C_TOT), dtype=np.float32)
    cst[:, C_W2:C_W2 + NBITS] = np.asarray(W2[:, :NBITS], dtype=np.float32)
    cst[:, C_TR:C_TR + NREP] = trep
    cst[:, C_WR:C_WR + NREP // 2] = np.ascontiguousarray(wrep).view(np.float32)

    bootw = np.zeros((128, BT_TOT), dtype=np.float32)
    bootw[:, BT_W1:BT_W1 + 64] = np.ascontiguousarray(w1b).view(np.float32)
    bootw[:, BT_B1] = np.asarray(b1, dtype=np.float32)
    return {"cst": cst, "bootw": bootw}


def make_bit_planes(virtual_addr):
    """Per-core [96, PER] bf16 0/1 bit planes (3x replicated)."""
    import ml_dtypes

    va32 = np.asarray(virtual_addr).astype(np.uint32)
    out = []
    for c in range(va32.size // PER):
        seg = va32[c * PER:(c + 1) * PER]
        bits = np.unpackbits(
            seg.view(np.uint8).reshape(-1, 4), axis=-1, bitorder="little"
        )  # [PER, 32]
        u16 = (bits.T.astype(np.uint16) * 0x3F80)  # [32, PER] bf16 bit pattern
        full = np.concatenate([u16, u16, u16], axis=0)  # [96, PER]
        out.append(np.ascontiguousarray(full).view(ml_dtypes.bfloat16))
    return out


def combine_output(o):
    """[NOUT, 128, OUTW] f32 -> [PER] int64."""
    res = np.empty(PER, dtype=np.int64)
    for t in range(N_ITERS):
        nch = SIZES[t] // MMCH
        off = OB_OFF[t]
        s = o[OB_OF[t], :, off:off + 2 * nch]       # [128, 2*nch]
        lo = s[:, 0::2].astype(np.int64)            # [128, nch]
        hi = s[:, 1::2].astype(np.int64)
        phys = (lo + 8192 * hi).T.reshape(-1)       # (chunk, partition) order
        res[CSTART[t]:CSTART[t + 1]] = phys
    return res


_NC_CACHE = {}
TRACE = False
LAST_RES = None


def kernel(virtual_addr, W1, b1, W2, b2):
    global LAST_RES
    if "nc" not in _NC_CACHE:
        nc = build_nc()
        nc.finalize()
        _NC_CACHE["nc"] = nc
    nc = _NC_CACHE["nc"]

    consts = make_const_inputs(W1, b1, W2, b2)
    planes = make_bit_planes(virtual_addr)
    in_maps = []
    for c in range(NCORES):
        boot = consts["bootw"].copy()
        boot[0:96, BT_BITS:BT_BITS + 256] = (
            np.ascontiguousarray(planes[c][:, :512]).view(np.float32)
        )
        in_maps.append({"bp": planes[c], "cst": consts["cst"], "boot": boot})

    res = bass_utils.run_bass_kernel_spmd(
        nc, in_maps, list(range(NCORES)), trace=TRACE
    )
    LAST_RES = res

    outs = [combine_output(res.results[c]["outp"]) for c in range(NCORES)]
    return np.concatenate(outs)


# revision 20
# speedup vs baseline: 1.0112x; 1.0000x over previous
"""NeuralMMU Trainium2 kernel, v2 — transposed second layer.

Per core (131072 addrs), 75 iterations sized [512, 1024, 36x(2048,1536),
512]: small first iterations so the first Gelu starts early, a small
final iteration so the pipeline drain is short, and alternating
2048/1536 in between (asymmetric PSUM tiles ringA 4 banks / ringB 3
banks + single-buffered pk in bank 8 — legal only because the deferred
L2 gives the pk WAR a full Gelu of slack).

  1. Host sends bit planes as bf16 [96, 131072] (bit k of addr a at
     partition k, replicated 3x for the 3-way bf16 split of W1). A
     single `boot` DMA carries W1+b1 plus iteration-0's bits so one
     DMA chain gates the first L1/Gelu; input groups are single
     iterations during ramp-up, pairs afterwards, prefetched ~6
     iterations ahead through 4 rotating buffers.
  2. L1: bf16 matmuls k=96 (512-addr blocks): bits @ (W1hi;W1mid;W1lo)
     -> PSUM hpre [128, <=1536] (exact: bits are 0/1, f32 accumulate).
  3. ACT Gelu(+b1): PSUM -> SBUF h f32, one instr/iter. This is the
     modeled bottleneck (~0.83 ns/elem + ~185 ns init per instr,
     ~126.6 us busy); everything else hides behind it.
  4. L2 TRANSPOSED: per 128-addr chunk, matmul with the h chunk
     [128 hid, 128 addr] as the *stationary* operand and W2[:, :26] f32
     as the *moving* operand -> PSUM pk [128 addr, nch*26] f32, exact,
     all chunks in one PSUM bank (start on first chunk, stop on last).
     Model cost 26*4 cyc/chunk vs 512*4 cyc per 512 addrs when h is the
     moving side: ~4.7x less PE time for the heavy layer. (LDWEIGHTS
     per chunk is free in the cost model and pipelined on silicon.)
  5. DVE: is_gt vs replicated thresholds (0.5 - b2[k]) -> bf16 bits,
     multiply by replicated 2^i weights (i = bit index within the lo/hi
     13-bit half), 4D tensor_reduce -> [128, 2*nch] f32 (lo, hi).
  6. Output batched 16 iters per DMA + a singleton final batch; host
     combines lo + 8192*hi.

PSUM: hpre 2 bufs x 3 banks + pk 2 bufs x 1 bank = 8 banks exactly.
(CHUNK=1664 by sharing pk into hpre's 4th bank was tried and is ~44 us
SLOWER: the bank's zero-region start=True forces a Tile dependency of
the next L1 on pk's DVE reader, serializing ACT behind DVE.)
Cost model (graded metric) 134.9 us vs 299.4 us baseline; correctness
runs on real silicon via PJRT (1/1048576 borderline mismatch, same as
the baseline, rel err 0.0065).
"""

import numpy as np
from contextlib import ExitStack

import concourse.bass as bass
import concourse.mybir as mybir
import concourse.tile as tile
from concourse import bacc, bass_utils

B = 1_048_576
NCORES = 8
PER = B // NCORES            # 131072 addrs per core
BLK = 512                    # addrs per L1 matmul block
CHUNK = 1536                 # max addrs per iteration (3 PSUM banks)
MMCH = 128                   # addrs per transposed L2 matmul
NBITS = 26

# Alternating sizes: even iterations use the 4-bank PSUM tile (<=2048
# addrs), odd the 3-bank one (<=1536); pk single-buffered in bank 8.
# Under the deferred-L2 schedule the pk WAR has a full Gelu of slack.
SIZES = [512, 1536] + [2048, 1536] * 34 + [1536, 1536, 1536, 1280, 1280]
assert sum(SIZES) == PER
N_ITERS = len(SIZES)
assert all(SIZES[t] <= (2048, 1536)[t % 2] for t in range(N_ITERS))
CSTART = [0]
for _s in SIZES:
    CSTART.append(CSTART[-1] + _s)

# Input DMA groups (lists of iterations): singles during ramp-up, pairs
# after. Iteration 0's bits ride in the boot tensor, not in a group.
GROUPS = ([[1], [2], [3], [4]] + [[i, i + 1] for i in range(5, 72, 2)]
          + [[73, 74]])
assert [t for g in GROUPS for t in g] == list(range(1, N_ITERS))
GRP_OF = {}
for _gi, _g in enumerate(GROUPS):
    for _t in _g:
        GRP_OF[_t] = _gi

# Output DMA batches: sixteen iterations each, then the tail alone so
# the final DMA after the last compute is tiny.
OBATCH = [list(range(r, min(r + 16, 74))) for r in range(0, 74, 16)] + [[74]]
OB_OF = {}
OB_OFF = {}
OB_USED = []
for _bi, _b in enumerate(OBATCH):
    used = 0
    for _t in _b:
        OB_OF[_t] = _bi
        OB_OFF[_t] = used
        used += 2 * (SIZES[_t] // MMCH)
    OB_USED.append(used)
NOUT = len(OBATCH)
OUTW = 28 * 16

F32 = mybir.dt.float32
BF16 = mybir.dt.bfloat16
AF = mybir.ActivationFunctionType
ALU = mybir.AluOpType
AX = mybir.AxisListType

# cst column layout (f32 columns); part A (w1b + b1) is DMA'd first so
# L1/Gelu can start before the larger part B arrives.
# boot tensor: w1b + b1 + iteration-0 bit planes, fetched in ONE DMA so a
# single 5-stage DMA chain gates the first L1/Gelu.
BT_W1 = 0         # [128, 64] f32 = [128, 128] bf16 3-way W1 split
BT_B1 = 64        # [128, 1] f32
BT_BITS = 65      # [96, 256] f32 = [96, 512] bf16 iteration-0 bits
BT_TOT = 65 + 256
# cst tensor (rest of the constants)
MAXCH = 16        # max transposed-L2 chunks per iteration
NREP = 26 * MAXCH  # 416 replicated threshold/weight columns
C_W2 = 0          # [128, 26] f32
C_TR = 26         # [128, 416] f32 thresholds replicated x16
C_WR = 442        # [128, 208] f32 = [128, 416] bf16 pack weights 2^i
C_TOT = 650


def build_nc() -> bass.Bass:
    nc = bacc.Bacc("TRN2")

    bp = nc.dram_tensor("bp", [96, PER], BF16, kind="ExternalInput")
    boot_d = nc.dram_tensor("boot", [128, BT_TOT], F32, kind="ExternalInput")
    cst_d = nc.dram_tensor("cst", [128, C_TOT], F32, kind="ExternalInput")
    outp = nc.dram_tensor("outp", [NOUT, 128, OUTW], F32, kind="ExternalOutput")

    with ExitStack() as ctx:
        tc = ctx.enter_context(tile.TileContext(nc))
        const = ctx.enter_context(tc.tile_pool(name="const", bufs=1))
        rpool = ctx.enter_context(tc.tile_pool(name="rp", bufs=4))
        hpre_p = ctx.enter_context(tc.tile_pool(name="hpre", bufs=1, space="PSUM"))
        hp = ctx.enter_context(tc.tile_pool(name="hp", bufs=3))
        pkp = ctx.enter_context(tc.tile_pool(name="pkp", bufs=1, space="PSUM"))
        yp = ctx.enter_context(tc.tile_pool(name="yp", bufs=2))
        zp = ctx.enter_context(tc.tile_pool(name="zp", bufs=2))
        so_p = ctx.enter_context(tc.tile_pool(name="so", bufs=2))

        boot = const.tile([128, BT_TOT], F32)
        nc.sync.dma_start(boot[:], boot_d[:])
        cst = const.tile([128, C_TOT], F32)

        w1b = boot[:, BT_W1:BT_W1 + 64].bitcast(BF16)    # [128, 128] bf16
        b1c = boot[:, BT_B1:BT_B1 + 1]
        bits0 = boot[0:96, BT_BITS:BT_BITS + 256].bitcast(BF16)  # [96, 512]
        w2s = cst[:, C_W2:C_W2 + NBITS]                  # [128, 26] f32
        trep = cst[:, C_TR:C_TR + NREP]                  # [128, 416] f32
        wrep = cst[:, C_WR:C_WR + NREP // 2].bitcast(BF16)  # [128, 416] bf16

        R = [None, None, None, None]
        next_group = 0

        def load_group(gi):
            g = GROUPS[gi]
            lo, hi = CSTART[g[0]], CSTART[g[-1] + 1]
            Rg = rpool.tile([96, 3584], BF16, name="Rg", tag="R")
            nc.sync.dma_start(Rg[:, : hi - lo], bp[:, lo:hi])
            R[gi % 4] = Rg

        def prefetch(upto_iter):
            nonlocal next_group
            while (next_group < len(GROUPS)
                   and GROUPS[next_group][0] <= upto_iter):
                load_group(next_group)
                next_group += 1

        def bits_of(t):
            gi = GRP_OF[t]
            off = CSTART[t] - CSTART[GROUPS[gi][0]]
            return R[gi % 4][0:96, off:off + SIZES[t]]

        ringA = hpre_p.tile([128, 2048], F32, name="ringA", tag="ringA")
        ringB = hpre_p.tile([128, 1536], F32, name="ringB", tag="ringB")
        rings = [ringA, ringB]

        def l1mm(t, bits=None):
            na = SIZES[t]
            hpre = rings[t % 2]
            if bits is None:
                bits = bits_of(t)
            for b in range(0, na, BLK):
                w = min(BLK, na - b)
                nc.tensor.matmul(
                    hpre[:, b:b + w],
                    w1b[0:96, :],
                    bits[:, b:b + w],
                    start=True, stop=True, tile_position=(0, 0),
                )
            return hpre[:, 0:na]

        # Startup: boot already issued; then bits for iterations 1-2, the
        # remaining constants, and iteration 3; in-loop prefetch takes over.
        prefetch(2)
        nc.sync.dma_start(cst[:], cst_d[:])
        prefetch(3)
        # Defer each iteration's entire transposed-L2 to the next
        # iteration's PE slot, and put L1(t+1) FIRST there: the in-order PE
        # queue then completes L1(t+1) ~880ns into Gelu(t)'s 1465ns window
        # instead of ~40ns after it (which cost a sem+decode stall on every
        # Gelu). h is triple-buffered so the deferred L2'(t-1) read never
        # collides with Gelu(t+1)'s output slot.
        def emit_l2(st, c0, c1):
            pk, h, nch = st["pk"], st["h"], st["nch"]
            for c in range(c0, min(c1, nch)):
                nc.tensor.matmul(
                    pk[:, NBITS * c:NBITS * (c + 1)],
                    h[:, MMCH * c:MMCH * (c + 1)],
                    w2s[:],
                    start=(c == 0), stop=(c == nch - 1),
                )

        def emit_dve(st, Sb):
            pk, nch, t = st["pk"], st["nch"], st["t"]
            ncol = NBITS * nch
            Y = yp.tile([128, NREP], BF16, name="Y", tag="Y")
            nc.vector.tensor_tensor(Y[:, :ncol], pk[:, :ncol], trep[:, :ncol],
                                    ALU.is_gt)
            Z = zp.tile([128, NREP], BF16, name="Z", tag="Z")
            nc.vector.tensor_tensor(Z[:, :ncol], Y[:, :ncol], wrep[:, :ncol],
                                    ALU.mult)
            if OB_OFF[t] == 0:
                Sb = so_p.tile([128, OUTW], F32, name="Sb", tag="S")
            z4 = Z[:, :ncol].rearrange("p (c h b) -> p c h b", c=nch, h=2, b=13)
            off = OB_OFF[t]
            nc.vector.tensor_reduce(Sb[:, off:off + 2 * nch], z4, AX.X, ALU.add)
            bi = OB_OF[t]
            if t == OBATCH[bi][-1]:
                used = OB_USED[bi]
                nc.sync.dma_start(outp[bi][:, :used], Sb[:, :used])
            return Sb

        hpre_cur = l1mm(0, bits=bits0)

        Sb = None
        prev = None
        for t in range(N_ITERS):
            na = SIZES[t]
            nch = na // MMCH

            # ACT: Gelu for iter t (waits L1(t))
            h = hp.tile([128, 2048], F32, name="h", tag="h")
            nc.scalar.activation(
                h[:, :na], hpre_cur, AF.Gelu, bias=b1c, scale=1.0
            )

            # PE: L1 for iter t+1 FIRST (only dep: Gelu(t-1) freed its
            # hpre slot, so it starts right away and finishes early)
            if t + 1 < N_ITERS:
                hpre_cur = l1mm(t + 1)

            # PE + DVE: previous iteration's full transposed L2 and pack
            if prev is not None:
                prev["pk"] = pkp.tile([128, NREP], F32, name="pk", tag="pk")
                emit_l2(prev, 0, prev["nch"])
                Sb = emit_dve(prev, Sb)
            cur = {"h": h, "nch": nch, "t": t}
            if t >= N_ITERS - 6:
                # Un-defer the last four iterations so the DVE backlog
                # (otherwise one full iteration deep, strict FIFO) drains
                # during the final Gelus and the last DMA starts earlier.
                # Each L1(t+1) is still queued ahead of the L2 batches, so
                # the trailing Gelus are not delayed.
                cur["pk"] = pkp.tile([128, NREP], F32, name="pk", tag="pk")
                emit_l2(cur, 0, cur["nch"])
                Sb = emit_dve(cur, Sb)
                prev = None
            else:
                prev = cur

            # Input prefetch ~6 iterations ahead. Issued after l1mm(t+1) so
            # the recycled R slot's readers are all already in the program.
            prefetch(t + 6)

        if prev is not None:
            prev["pk"] = pkp.tile([128, NREP], F32, name="pk", tag="pk")
            emit_l2(prev, 0, prev["nch"])
            Sb = emit_dve(prev, Sb)

    return nc


def make_const_inputs(W1, b1, W2, b2):
    import ml_dtypes

    w1 = np.ascontiguousarray(W1[0:32, :], dtype=np.float32)
    hi = w1.astype(ml_dtypes.bfloat16)
    mid = (w1 - hi.astype(np.float32)).astype(ml_dtypes.bfloat16)
    lo = (w1 - hi.astype(np.float32) - mid.astype(np.float32)).astype(
        ml_dtypes.bfloat16
    )
    w1b = np.zeros((128, 128), dtype=ml_dtypes.bfloat16)
    w1b[0:32] = hi
    w1b[32:64] = mid
    w1b[64:96] = lo

    thr = (0.5 - np.asarray(b2[:NBITS], dtype=np.float32))  # [26]
    trep = np.tile(thr, MAXCH)[None, :].repeat(128, axis=0)  # [128, 416]

    wvec = np.zeros(NREP, dtype=np.float32)
    for c in range(MAXCH):
        for h in range(2):
            for i in range(13):
                wvec[26 * c + 13 * h + i] = float(1 << i)
    wrep = wvec[None, :].repeat(128, axis=0).astype(ml_dtypes.bfloat16)

    cst = np.zeros((128, C_TOT), dtype=np.float32)
    cst[:, C_W2:C_W2 + NBITS] = np.asarray(W2[:, :NBITS], dtype=np.float32)
    cst[:, C_TR:C_TR + NREP] = trep
    cst[:, C_WR:C_WR + NREP // 2] = np.ascontiguousarray(wrep).view(np.float32)

    bootw = np.zeros((128, BT_TOT), dtype=np.float32)
    bootw[:, BT_W1:BT_W1 + 64] = np.ascontiguousarray(w1b).view(np.float32)
    bootw[:, BT_B1] = np.asarray(b1, dtype=np.float32)
    return {"cst": cst, "bootw": bootw}


def make_bit_planes(virtual_addr):
    """Per-core [96, PER] bf16 0/1 bit planes (3x replicated)."""
    import ml_dtypes

    va32 = np.asarray(virtual_addr).astype(np.uint32)
    out = []
    for c in range(va32.size // PER):
        seg = va32[c * PER:(c + 1) * PER]
        bits = np.unpackbits(
            seg.view(np.uint8).reshape(-1, 4), axis=-1, bitorder="little"
        )  # [PER, 32]
        u16 = (bits.T.astype(np.uint16) * 0x3F80)  # [32, PER] bf16 bit pattern
        full = np.concatenate([u16, u16, u16], axis=0)  # [96, PER]
        out.append(np.ascontiguousarray(full).view(ml_dtypes.bfloat16))
    return out


def combine_output(o):
    """[NOUT, 128, OUTW] f32 -> [PER] int64."""
    res = np.empty(PER, dtype=np.int64)
    for t in range(N_ITERS):
        nch = SIZES[t] // MMCH
        off = OB_OFF[t]
        s = o[OB_OF[t], :, off:off + 2 * nch]       # [128, 2*nch]
        lo = s[:, 0::2].astype(np.int64)            # [128, nch]
        hi = s[:, 1::2].astype(np.int64)
        phys = (lo + 8192 * hi).T.reshape(-1)       # (chunk, partition) order
        res[CSTART[t]:CSTART[t + 1]] = phys
    return res


_NC_CACHE = {}
TRACE = False
LAST_RES = None


def kernel(virtual_addr, W1, b1, W2, b2):
    global LAST_RES
    if "nc" not in _NC_CACHE:
        nc = build_nc()
        nc.finalize()
        _NC_CACHE["nc"] = nc
    nc = _NC_CACHE["nc"]

    consts = make_const_inputs(W1, b1, W2, b2)
    planes = make_bit_planes(virtual_addr)
    in_maps = []
    for c in range(NCORES):
        boot = consts["bootw"].copy()
        boot[0:96, BT_BITS:BT_BITS + 256] = (
            np.ascontiguousarray(planes[c][:, :512]).view(np.float32)
        )
        in_maps.append({"bp": planes[c], "cst": consts["cst"], "boot": boot})

    res = bass_utils.run_bass_kernel_spmd(
        nc, in_maps, list(range(NCORES)), trace=TRACE
    )
    LAST_RES = res

    outs = [combine_output(res.results[c]["outp"]) for c in range(NCORES)]
    return np.concatenate(outs)
